# revision 41
# baseline (speedup 1.0000x reference)
"""Trainium2 Bass kernel for nn_Attention (T=2048, D=2048, H=16, Dh=128).

Strategy: tensor-parallel over heads, 2 heads per core on 8 cores.
  - host folds w_ln into wq/wk/wv, splits x and the (WS=64-scaled) weights
    into fp8(e4m3) hi/lo residual pairs, precomputes fp8 x^2, and folds the
    rotate_half sign into the sin table
  - device: q/k/v projections as 3-shot residual-fp8 DoubleRow matmuls
    (hi@hi + lo@hi + hi@lo over kd-pairs, 0.75x the bf16 cycles); RMSNorm
    sums as DoubleRow fp8 ones-matmuls over the hosted x^2; projection
    PSUMs evacuated to bf16 SBUF on ACT immediately so PSUM banks recycle
    at PE speed; RoPE on DVE from the evacs; causal mask folded into the
    score PSUM via a step-matrix matmul; softmax normalization deferred to
    the PV epilogue via exp(-ln(sum)); per-head output projection in PSUM,
    staged to bf16 and DMA'd out
  - host sums the 8 bf16 partial outputs in f32 and adds the residual x.
"""

import math
import os
import sys
import time

for _p in ("/opt/trn_rl_repo", "/root/.axon_site/_ro/trn_rl_repo"):
    if os.path.isdir(_p) and _p not in sys.path:
        sys.path.insert(0, _p)

import numpy as np
import ml_dtypes

import concourse.bass as bass
import concourse.tile as tile
from concourse import bacc, mybir
from concourse.bass_utils import run_bass_kernel_spmd

BF16 = mybir.dt.bfloat16
FP8 = mybir.dt.float8e4
F32R = mybir.dt.float32r
F32 = mybir.dt.float32
AF = mybir.ActivationFunctionType
DR = mybir.MatmulPerfMode.DoubleRow

T = 2048
D = 2048
N_H = 16
D_H = 128
N_CORES = 8
H_LOC = N_H // N_CORES          # heads per core = 2
NL = H_LOC * D_H                # local head width = 256
KD = D // 128                   # contraction tiles = 16
TT = T // 128                   # t tiles = 16
NS = T // 512                   # 512-wide strips = 4
EPS = 1e-5
INV_SQRT_DH = 1.0 / math.sqrt(D_H)
WS = 64.0                       # fp8 weight scale (undone via the s tables)

_CACHED = {}


def _build_program(repeats=1):
    if repeats in _CACHED:
        return _CACHED[repeats]

    nc = bacc.Bacc("TRN2", target_bir_lowering=False, debug=False, num_devices=N_CORES)

    xh_d = nc.dram_tensor("xh", [D, T], FP8, kind="ExternalInput")
    xl_d = nc.dram_tensor("xl", [D, T], FP8, kind="ExternalInput")
    sq_d = nc.dram_tensor("sq8", [D // 4, T], FP8, kind="ExternalInput")
    w_d = {}
    for nm in ("wqh", "wql", "wkh", "wkl", "wvh", "wvl"):
        w_d[nm] = nc.dram_tensor(nm, [D, NL], FP8, kind="ExternalInput")
    woh_d = nc.dram_tensor("woh", [NL, T], FP8, kind="ExternalInput")
    wol_d = nc.dram_tensor("wol", [NL, T], FP8, kind="ExternalInput")
    cos_d = nc.dram_tensor("cosT", [D_H, T], BF16, kind="ExternalInput")
    sin_d = nc.dram_tensor("sinT", [D_H, T], BF16, kind="ExternalInput")
    step_d = nc.dram_tensor("stepm", [128, 128], BF16, kind="ExternalInput")
    negi_d = nc.dram_tensor("negim", [128, 128], BF16, kind="ExternalInput")
    pswp_d = nc.dram_tensor("pswap", [128, 128], BF16, kind="ExternalInput")
    onc_d = nc.dram_tensor("ones_col", [1, 128], F32R, kind="ExternalInput")
    on128_d = nc.dram_tensor("ones128", [128, 1], BF16, kind="ExternalInput")
    on8_d = nc.dram_tensor("ones8", [128, 2, 32], FP8, kind="ExternalInput")
    out_d = nc.dram_tensor("out", [T, D], BF16, kind="ExternalOutput")
    # DRAM scratch for the s row->tile-layout round trip
    s_scr = nc.dram_tensor("s_scr", [TT, 128], F32, kind="Internal")

    ap = lambda h: h.ap()
    out_ap, s_scr_ap = ap(out_d), ap(s_scr)

    from contextlib import ExitStack

    with tile.TileContext(nc) as tc, ExitStack() as ctx:
        P = ctx.enter_context  # noqa

        singles = P(tc.tile_pool(name="singles", bufs=1))
        rope = P(tc.tile_pool(name="rope", bufs=4))        # [128,512] bf16
        qke = P(tc.tile_pool(name="qke", bufs=4))          # [128,512] bf16 qk evac
        epool = P(tc.tile_pool(name="epool", bufs=6))      # [128,512] bf16 exp tiles
        rbsp = P(tc.tile_pool(name="rbsp", bufs=2))        # [128,512] bf16 pv evac
        small = P(tc.tile_pool(name="small", bufs=2))      # [1,512] smalls
        stage = P(tc.tile_pool(name="stage", bufs=6))      # [128,512] bf16 out staging
        pmm = P(tc.tile_pool(name="pmm", bufs=3, space="PSUM"))
        psc = P(tc.tile_pool(name="psc", bufs=3, space="PSUM"))
        ppv = P(tc.tile_pool(name="ppv", bufs=1, space="PSUM"))
        psums = P(tc.tile_pool(name="psums", bufs=1, space="PSUM"))

        def emit_body():
            # ---------------- phase 0: loads ----------------------------------
            onc = singles.tile([1, 128], F32R, tag="onc")
            nc.sync.dma_start(out=onc, in_=ap(onc_d))
            on128 = singles.tile([128, 1], BF16, tag="on128")
            nc.sync.dma_start(out=on128, in_=ap(on128_d))
            on8 = singles.tile([128, 2, 32], FP8, tag="on8")
            nc.sync.dma_start(out=on8, in_=ap(on8_d))
            stepm = singles.tile([128, 128], BF16, tag="stepm")
            nc.sync.dma_start(out=stepm, in_=ap(step_d))
            negim = singles.tile([128, 128], BF16, tag="negim")
            nc.sync.dma_start(out=negim, in_=ap(negi_d))
            pswp = singles.tile([128, 128], BF16, tag="pswp")
            nc.sync.dma_start(out=pswp, in_=ap(pswp_d))
            epsb = singles.tile([1, 1], F32, tag="epsb")
            nc.vector.memset(epsb, EPS)
            lnws = singles.tile([1, 1], F32, tag="lnws")
            nc.vector.memset(lnws, -math.log(WS))
            s_row = singles.tile([1, T], F32R, tag="srow")
            cos_s = singles.tile([128, T], BF16, tag="cos_s")
            sin_s = singles.tile([128, T], BF16, tag="sin_s")
            sk_t = singles.tile([128, TT], F32, tag="sk")

            sq = singles.tile([128, KD // 4, T], FP8, tag="sq")
            xh = singles.tile([128, KD, T], FP8, tag="xh")
            xl = singles.tile([128, KD, T], FP8, tag="xl")
            sqv = ap(sq_d).rearrange("(n p) t -> p n t", p=128)
            xhv = ap(xh_d).rearrange("(n p) t -> p n t", p=128)
            xlv = ap(xl_d).rearrange("(n p) t -> p n t", p=128)

            def load_w(nm, tag):
                t_ = singles.tile([128, KD, NL], FP8, tag=tag)
                nc.sync.dma_start(out=t_, in_=ap(w_d[nm]).rearrange("(a p) m -> p a m", p=128))
                return t_

            # folded x^2 first (2 x 0.5MB) so the ssq matmuls start early; then
            # the q/k weights, then xh (ACT queue) / xl (SP queue) in 1MB chunks.
            for i in range(2):
                sl = slice(2 * i, 2 * i + 2)
                nc.scalar.dma_start(out=sq[:, sl, :], in_=sqv[:, sl, :])
            wqh, wkh = load_w("wqh", "wqh"), load_w("wkh", "wkh")
            wql, wkl = load_w("wql", "wql"), load_w("wkl", "wkl")
            for i in range(8):
                sl = slice(2 * i, 2 * i + 2)
                nc.scalar.dma_start(out=xh[:, sl, :], in_=xhv[:, sl, :])
                nc.sync.dma_start(out=xl[:, sl, :], in_=xlv[:, sl, :])
            wvh, wvl = load_w("wvh", "wvh"), load_w("wvl", "wvl")
            woh = singles.tile([128, H_LOC, T], FP8, tag="woh")
            nc.sync.dma_start(out=woh, in_=ap(woh_d).rearrange("(h p) t -> p h t", p=128))
            wol = singles.tile([128, H_LOC, T], FP8, tag="wol")
            nc.sync.dma_start(out=wol, in_=ap(wol_d).rearrange("(h p) t -> p h t", p=128))

            # ---------------- phase 3: projections + attention ----------------
            q_sb = singles.tile([128, H_LOC, T], BF16, tag="q_sb")
            k_sb = singles.tile([128, H_LOC, T], BF16, tag="k_sb")
            v_sb = singles.tile([128, TT, NL], BF16, tag="v_sb")
            vT_sb = singles.tile([128, H_LOC, T], BF16, tag="outT", name="vT_sb")
            outT = singles.tile([128, H_LOC, T], BF16, tag="outT")
            oth = singles.tile([128, H_LOC, T], FP8, tag="oth")
            otl = singles.tile([128, H_LOC, T], FP8, tag="otl")

            def proj_group(ps, wh, wl, hs, js):
                # 3-shot residual fp8 DoubleRow; hi shots first (xl lands last)
                for sh, (wa, xb) in enumerate(((wh, xh), (wl, xh), (wh, xl))):
                    for i in range(KD // 2):
                        kp = slice(2 * i, 2 * i + 2)
                        nc.tensor.matmul(ps, lhsT=wa[:, kp, hs], rhs=xb[:, kp, js],
                                         start=(sh == 0 and i == 0),
                                         stop=(sh == 2 and i == KD // 2 - 1),
                                         perf_mode=DR)

            def emit_vT_strip(h, j):
                hs = slice(h * 128, (h + 1) * 128)
                js = slice(j * 512, (j + 1) * 512)
                ps = pmm.tile([128, 512], F32, tag="mm")
                proj_group(ps, wvh, wvl, hs, js)
                if j % 2 == 0:
                    nc.vector.tensor_copy(vT_sb[:, h, js], ps)
                else:
                    nc.scalar.copy(vT_sb[:, h, js], ps)

            def emit_v_finish(h):
                hs = slice(h * 128, (h + 1) * 128)
                nc.sync.dma_start_transpose(v_sb[:, :, hs], vT_sb[:, h, :])
                for tt in range(TT):
                    nc.vector.tensor_scalar_mul(
                        v_sb[:, tt, hs], v_sb[:, tt, hs], sk_t[:, tt : tt + 1]
                    )

            def emit_rope(ev, dst, h, js):
                # dh-half swap via a PE permutation matmul (PSUM out), so the
                # sin mul is a legal PSUM x SBUF op; sign lives in the table
                m1 = rope.tile([128, 512], BF16, tag="m1")
                nc.vector.tensor_mul(m1, ev, cos_s[:, js])
                msw = psc.tile([128, 512], F32, tag="sc")
                nc.tensor.matmul(msw, lhsT=pswp, rhs=ev, start=True, stop=True)
                m2 = rope.tile([128, 512], BF16, tag="m2")
                nc.vector.tensor_mul(m2, msw, sin_s[:, js])
                nc.vector.tensor_add(dst[:, h, js], m1, m2)

            def emit_qk_group(h, dst, wh, wl, j):
                # projection + immediate ACT evac (frees the PSUM bank fast)
                ps = pmm.tile([128, 512], F32, tag="mm")
                proj_group(ps, wh, wl, slice(h * 128, (h + 1) * 128),
                           slice(j * 512, (j + 1) * 512))
                ev = qke.tile([128, 512], BF16, tag="qke")
                nc.scalar.copy(ev, ps)
                return ev

            def emit_qk_strip(h, j):
                js = slice(j * 512, (j + 1) * 512)
                for dst, wh, wl in ((k_sb, wkh, wkl), (q_sb, wqh, wql)):
                    ev = emit_qk_group(h, dst, wh, wl, j)
                    emit_rope(ev, dst, h, js)

            def emit_attention(h, j):
                hs = slice(h * 128, (h + 1) * 128)
                js = slice(j * 512, (j + 1) * 512)
                ntk = 4 * (j + 1)
                po = ppv.tile([128, 512], F32, tag="pv")
                su = psums.tile([1, 512], F32, tag="su")
                for i in range(ntk):
                    # causal trim: diagonal tile r = i-4j only has valid scores in
                    # columns f >= 128r; restrict every matmul/exp to that range.
                    r = i - 4 * j
                    c0 = 128 * r if r >= 0 else 0
                    cs = slice(c0, 512)           # kept tq columns within strip
                    qs = slice(j * 512 + c0, (j + 1) * 512)
                    st = psc.tile([128, 512], F32, tag="sc")
                    nc.tensor.matmul(
                        st[:, cs], lhsT=k_sb[:, h, i * 128 : (i + 1) * 128],
                        rhs=q_sb[:, h, qs], start=True, stop=(r < 0),
                        skip_group_check=True,
                    )
                    if r >= 0:
                        # causal boundary: -1e30 above the diagonal via a
                        # step-matrix matmul accumulated into the same PSUM
                        nc.tensor.matmul(
                            st[:, c0 : c0 + 128], lhsT=stepm, rhs=negim,
                            start=False, stop=True, skip_group_check=True,
                        )
                    e = epool.tile([128, 512], BF16, tag="e")
                    nc.scalar.activation(e[:, cs], st[:, cs], AF.Exp, scale=INV_SQRT_DH)
                    nc.tensor.matmul(
                        po[:, cs], lhsT=v_sb[:, i, hs], rhs=e[:, cs],
                        start=(i == 0), stop=(i == ntk - 1),
                    )
                    nc.tensor.matmul(
                        su[:, cs], lhsT=on128, rhs=e[:, cs],
                        start=(i == 0), stop=(i == ntk - 1),
                    )
                pos = rbsp.tile([128, 512], BF16, tag="pos")
                nc.vector.tensor_copy(pos, po)
                lnr = small.tile([1, 512], F32, tag="lnr")
                nc.scalar.activation(lnr, su, AF.Ln)
                rec = small.tile([1, 512], F32R, tag="rec")
                nc.scalar.activation(rec, lnr, AF.Exp, scale=-1.0)
                rb = psums.tile([128, 512], F32, tag="su")
                nc.tensor.matmul(rb, lhsT=onc, rhs=rec, start=True, stop=True)
                nc.vector.tensor_mul(outT[:, h, js], rb, pos)
                # residual fp8 split of outT for the DoubleRow wo projection
                nc.vector.tensor_copy(oth[:, h, js], outT[:, h, js])
                nc.vector.tensor_sub(otl[:, h, js], outT[:, h, js], oth[:, h, js])

            def emit_wo_tile(tt):
                ts = slice(tt * 128, (tt + 1) * 128)
                for n in range(NS):
                    ns = slice(n * 512, (n + 1) * 512)
                    ps = pmm.tile([128, 512], F32, tag="mm")
                    for sh, (oa, wb) in enumerate(((oth, woh), (otl, woh), (oth, wol))):
                        nc.tensor.matmul(
                            ps, lhsT=oa[:, :, ts], rhs=wb[:, :, ns],
                            start=(sh == 0), stop=(sh == 2), perf_mode=DR,
                        )
                    stg = stage.tile([128, 512], BF16, tag="stg")
                    if n % 2 == 0:
                        nc.vector.tensor_copy(stg, ps)
                    else:
                        nc.scalar.copy(stg, ps)
                    nc.sync.dma_start(out=out_ap[ts, ns], in_=stg)

            # emission order = scheduler priority: feed PE the next strip's q/k/v
            # before the current strip's epilogue so PE never waits on ACT/DVE.
            # ---------------- phase 1: ssq via DoubleRow ones-matmuls ---------
            ssqp = [psc.tile([32, 512], F32, tag="sc", name=f"ssq{i}") for i in range(2)] + \
                   [pmm.tile([32, 512], F32, tag="mm", name=f"ssq{i+2}") for i in range(2)]
            for i in range(KD // 8):
                kp = slice(2 * i, 2 * i + 2)
                for j in range(NS):
                    nc.tensor.matmul(
                        ssqp[j], lhsT=on8, rhs=sq[:, kp, j * 512 : (j + 1) * 512],
                        start=(i == 0), stop=(i == KD // 8 - 1), perf_mode=DR,
                    )

            combos0 = [(h, dst, wh, wl) for h in range(H_LOC)
                       for dst, wh, wl in ((k_sb, wkh, wkl), (q_sb, wqh, wql))]
            g0 = [(emit_qk_group(h, dst, wh, wl, 0), dst, h)
                  for h, dst, wh, wl in combos0[:3]]
            # ---------------- phase 2: s chain + tables -----------------------
            for j in range(NS):
                js = slice(j * 512, (j + 1) * 512)
                s_m = small.tile([1, 512], F32, tag="lnr")
                nc.vector.tensor_copy(s_m, ssqp[j][0:1, :])
                nc.scalar.activation(s_m, s_m, AF.Ln, bias=epsb, scale=1.0 / D)
                nc.scalar.activation(s_row[:, js], s_m, AF.Exp, bias=lnws, scale=-0.5)
                sb = psc.tile([128, 512], F32, tag="sc")
                nc.tensor.matmul(sb, lhsT=onc, rhs=s_row[:, js], start=True, stop=True)
                cstrip = rope.tile([128, 512], BF16, tag="m1")
                nc.sync.dma_start(out=cstrip, in_=ap(cos_d)[:, js])
                nc.vector.tensor_mul(cos_s[:, js], cstrip, sb)
                sstrip = rope.tile([128, 512], BF16, tag="m2")
                nc.sync.dma_start(out=sstrip, in_=ap(sin_d)[:, js])
                nc.vector.tensor_mul(sin_s[:, js], sstrip, sb)
            # round-trip for the [128, TT] t-tile layout (v scaling)
            nc.sync.dma_start(out=s_scr_ap.rearrange("i p -> () (i p)"), in_=s_row.bitcast(F32))
            nc.sync.dma_start(out=sk_t, in_=s_scr_ap.rearrange("i p -> p i"))

            h4, dst4, wh4, wl4 = combos0[3]
            g0.append((emit_qk_group(h4, dst4, wh4, wl4, 0), dst4, h4))
            for ev, dst, h in g0:
                emit_rope(ev, dst, h, slice(0, 512))
            for h in range(H_LOC):
                for j in range(NS):
                    emit_vT_strip(h, j)
                emit_v_finish(h)
            for j in range(NS):
                for h in range(H_LOC):
                    emit_attention(h, j)
                if j + 1 < NS:
                    for h in range(H_LOC):
                        emit_qk_strip(h, j + 1)
                for tt in range(4 * j, 4 * (j + 1)):
                    emit_wo_tile(tt)

        for _rep in range(repeats):
            emit_body()

    # Force Exp and Ln onto the single combined table set: drop them from
    # every other set in the (cached, order-preserving) table map so the
    # table-load pass picks natural_log_exp_and_others for both — one
    # ACT_TABLE_LOAD for the whole kernel instead of per-strip thrash.
    from concourse.hw_specs import get_activation_tables
    tabs = get_activation_tables(nc.m.arch)
    for nm_, fs_ in tabs.items():
        if nm_ != "natural_log_exp_and_others":
            fs_.discard(AF.Exp)
            fs_.discard(AF.Ln)
    nc.compile()
    _CACHED[repeats] = nc
    return nc


def _host_prep(x, w_ln, wq, wk, wv, wo, cos, sin):
    bf = ml_dtypes.bfloat16
    f8 = ml_dtypes.float8_e4m3
    x = np.asarray(x, np.float32)
    w_ln = np.asarray(w_ln, np.float32)
    cosT = np.ascontiguousarray(np.asarray(cos, np.float32).T).astype(bf)
    sinTf = np.ascontiguousarray(np.asarray(sin, np.float32).T)
    sinTf[0:64] *= -1.0          # rotate_half sign folded into the table
    sinT = sinTf.astype(bf)

    xT = np.ascontiguousarray(x.T)
    xh = xT.astype(f8)
    xl = (xT - xh.astype(np.float32)).astype(f8)
    sq8 = (xT.astype(np.float32) ** 2).reshape(D // 4, 4, T).sum(axis=1).astype(f8)

    # causal step/neg tiles: step.T @ negI adds -1e30 where tq < tk
    p = np.arange(128)
    stepm = (p[:, None] < p[None, :]).astype(bf)
    pswap = np.zeros((128, 128), np.float32)
    pswap[(p + 64) % 128, p] = 1.0          # mswap[i] = ev[(i+64)%128]
    pswap = pswap.astype(bf)
    negim = (-1e30 * np.eye(128)).astype(bf)
    ones_col = np.ones((1, 128), np.float32)
    ones128 = np.ones((128, 1), bf)
    ones8 = np.ones((128, 2, 32), f8)

    wq_s = (np.asarray(wq, np.float32) * w_ln[None, :]) * WS
    wk_s = (np.asarray(wk, np.float32) * w_ln[None, :]) * WS
    wv_s = (np.asarray(wv, np.float32) * w_ln[None, :]) * WS
    wo32 = np.asarray(wo, np.float32)

    def split8(w):                 # [NL, D] f32 -> hi, lo fp8 (transposed)
        wT = np.ascontiguousarray(w.T)
        hi = wT.astype(f8)
        lo = (wT - hi.astype(np.float32)).astype(f8)
        return hi, lo

    in_maps = []
    for c in range(N_CORES):
        sl = slice(c * NL, (c + 1) * NL)
        wqh, wql = split8(wq_s[sl])
        wkh, wkl = split8(wk_s[sl])
        wvh, wvl = split8(wv_s[sl])
        woT = np.ascontiguousarray(wo32[:, sl].T) * WS
        woh_ = woT.astype(f8)
        wol_ = (woT - woh_.astype(np.float32)).astype(f8)
        in_maps.append({
            "xh": xh, "xl": xl, "sq8": sq8,
            "wqh": wqh, "wql": wql,
            "wkh": wkh, "wkl": wkl,
            "wvh": wvh, "wvl": wvl,
            "woh": woh_, "wol": wol_,
            "cosT": cosT, "sinT": sinT,
            "stepm": stepm, "negim": negim, "pswap": pswap,
            "ones_col": ones_col, "ones128": ones128, "ones8": ones8,
        })
    return in_maps


def kernel(x, w_ln, wq, wk, wv, wo, cos, sin):
    nc = _build_program()
    in_maps = _host_prep(x, w_ln, wq, wk, wv, wo, cos, sin)
    t0 = time.time()
    res = run_bass_kernel_spmd(nc, in_maps, core_ids=list(range(N_CORES)))
    t1 = time.time()
    print(f"run_bass_kernel_spmd wall: {(t1 - t0) * 1e3:.1f} ms", file=sys.stderr)
    acc = np.zeros((T, D), np.float32)
    for r in res.results:
        acc += r["out"].astype(np.float32) / WS
    return np.asarray(x, np.float32) + acc


# revision 58
# speedup vs baseline: 1.0013x; 1.0013x over previous
"""Trainium2 Bass kernel for nn_Attention (T=2048, D=2048, H=16, Dh=128).

Strategy: tensor-parallel over heads, 2 heads per core on 8 cores.
  - host folds w_ln into wq/wk/wv, splits x and the (WS=64-scaled) weights
    into fp8(e4m3) hi/lo residual pairs, precomputes fp8 x^2, and folds the
    rotate_half sign into the sin table
  - device: q/k/v projections as 3-shot residual-fp8 DoubleRow matmuls
    (hi@hi + lo@hi + hi@lo over kd-pairs, 0.75x the bf16 cycles); RMSNorm
    sums as DoubleRow fp8 ones-matmuls over the hosted x^2; projection
    PSUMs evacuated to bf16 SBUF on ACT immediately so PSUM banks recycle
    at PE speed; RoPE on DVE from the evacs; causal mask folded into the
    score PSUM via a step-matrix matmul; softmax normalization deferred to
    the PV epilogue via exp(-ln(sum)); per-head output projection in PSUM,
    staged to bf16 and DMA'd out
  - host sums the 8 bf16 partial outputs in f32 and adds the residual x.
"""

import math
import os
import sys
import time

for _p in ("/opt/trn_rl_repo", "/root/.axon_site/_ro/trn_rl_repo"):
    if os.path.isdir(_p) and _p not in sys.path:
        sys.path.insert(0, _p)

import numpy as np
import ml_dtypes

import concourse.bass as bass
import concourse.tile as tile
from concourse import bacc, mybir
from concourse.bass_utils import run_bass_kernel_spmd

BF16 = mybir.dt.bfloat16
FP8 = mybir.dt.float8e4
F32R = mybir.dt.float32r
F32 = mybir.dt.float32
AF = mybir.ActivationFunctionType
DR = mybir.MatmulPerfMode.DoubleRow

T = 2048
D = 2048
N_H = 16
D_H = 128
N_CORES = 8
H_LOC = N_H // N_CORES          # heads per core = 2
NL = H_LOC * D_H                # local head width = 256
KD = D // 128                   # contraction tiles = 16
TT = T // 128                   # t tiles = 16
NS = T // 512                   # 512-wide strips = 4
EPS = 1e-5
INV_SQRT_DH = 1.0 / math.sqrt(D_H)
WS = 64.0                       # fp8 weight scale (undone via the s tables)

_CACHED = {}


def _build_program(repeats=1):
    if repeats in _CACHED:
        return _CACHED[repeats]

    nc = bacc.Bacc("TRN2", target_bir_lowering=False, debug=False, num_devices=N_CORES)

    xh_d = nc.dram_tensor("xh", [D, T], FP8, kind="ExternalInput")
    xl_d = nc.dram_tensor("xl", [D, T], FP8, kind="ExternalInput")
    sq_d = nc.dram_tensor("sq8", [D // 4, T], FP8, kind="ExternalInput")
    w_d = {}
    for nm in ("wqh", "wql", "wkh", "wkl", "wvh", "wvl"):
        w_d[nm] = nc.dram_tensor(nm, [D, NL], FP8, kind="ExternalInput")
    woh_d = nc.dram_tensor("woh", [NL, T], FP8, kind="ExternalInput")
    wol_d = nc.dram_tensor("wol", [NL, T], FP8, kind="ExternalInput")
    cos_d = nc.dram_tensor("cosT", [D_H, T], BF16, kind="ExternalInput")
    sin_d = nc.dram_tensor("sinT", [D_H, T], BF16, kind="ExternalInput")
    step_d = nc.dram_tensor("stepm", [128, 128], BF16, kind="ExternalInput")
    negi_d = nc.dram_tensor("negim", [128, 128], BF16, kind="ExternalInput")
    pswp_d = nc.dram_tensor("pswap", [128, 128], BF16, kind="ExternalInput")
    onc_d = nc.dram_tensor("ones_col", [1, 128], F32R, kind="ExternalInput")
    on128_d = nc.dram_tensor("ones128", [128, 1], BF16, kind="ExternalInput")
    on8_d = nc.dram_tensor("ones8", [128, 2, 32], FP8, kind="ExternalInput")
    out_d = nc.dram_tensor("out", [T, D], BF16, kind="ExternalOutput")
    # DRAM scratch for the s row->tile-layout round trip
    s_scr = nc.dram_tensor("s_scr", [TT, 128], F32, kind="Internal")

    ap = lambda h: h.ap()
    out_ap, s_scr_ap = ap(out_d), ap(s_scr)

    from contextlib import ExitStack

    with tile.TileContext(nc) as tc, ExitStack() as ctx:
        P = ctx.enter_context  # noqa

        singles = P(tc.tile_pool(name="singles", bufs=1))
        rope = P(tc.tile_pool(name="rope", bufs=4))        # [128,512] bf16
        qke = P(tc.tile_pool(name="qke", bufs=4))          # [128,512] bf16 qk evac
        epool = P(tc.tile_pool(name="epool", bufs=7))      # [128,512] bf16 exp tiles
        rbsp = P(tc.tile_pool(name="rbsp", bufs=2))        # [128,512] bf16 pv evac
        small = P(tc.tile_pool(name="small", bufs=2))      # [1,512] smalls
        stage = P(tc.tile_pool(name="stage", bufs=6))      # [128,512] bf16 out staging
        pmm = P(tc.tile_pool(name="pmm", bufs=3, space="PSUM"))
        psc = P(tc.tile_pool(name="psc", bufs=3, space="PSUM"))
        ppv = P(tc.tile_pool(name="ppv", bufs=1, space="PSUM"))
        psums = P(tc.tile_pool(name="psums", bufs=1, space="PSUM"))

        def emit_body():
            # ---------------- phase 0: loads ----------------------------------
            onc = singles.tile([1, 128], F32R, tag="onc")
            nc.sync.dma_start(out=onc, in_=ap(onc_d))
            on128 = singles.tile([128, 1], BF16, tag="on128")
            nc.sync.dma_start(out=on128, in_=ap(on128_d))
            on8 = singles.tile([128, 2, 32], FP8, tag="on8")
            nc.sync.dma_start(out=on8, in_=ap(on8_d))
            stepm = singles.tile([128, 128], BF16, tag="stepm")
            nc.sync.dma_start(out=stepm, in_=ap(step_d))
            negim = singles.tile([128, 128], BF16, tag="negim")
            nc.sync.dma_start(out=negim, in_=ap(negi_d))
            pswp = singles.tile([128, 128], BF16, tag="pswp")
            nc.sync.dma_start(out=pswp, in_=ap(pswp_d))
            epsb = singles.tile([1, 1], F32, tag="epsb")
            nc.vector.memset(epsb, EPS)
            lnws = singles.tile([1, 1], F32, tag="lnws")
            nc.vector.memset(lnws, -math.log(WS))
            s_row = singles.tile([1, T], F32R, tag="srow")
            cos_s = singles.tile([128, T], BF16, tag="cos_s")
            sin_s = singles.tile([128, T], BF16, tag="sin_s")
            sk_t = singles.tile([128, TT], F32, tag="sk")

            sq = singles.tile([128, KD // 4, T], FP8, tag="sq")
            xh = singles.tile([128, KD, T], FP8, tag="xh")
            xl = singles.tile([128, KD, T], FP8, tag="xl")
            sqv = ap(sq_d).rearrange("(n p) t -> p n t", p=128)
            xhv = ap(xh_d).rearrange("(n p) t -> p n t", p=128)
            xlv = ap(xl_d).rearrange("(n p) t -> p n t", p=128)

            def load_w(nm, tag):
                t_ = singles.tile([128, KD, NL], FP8, tag=tag)
                nc.sync.dma_start(out=t_, in_=ap(w_d[nm]).rearrange("(a p) m -> p a m", p=128))
                return t_

            # folded x^2 first (2 x 0.5MB) so the ssq matmuls start early; then
            # the q/k weights, then xh (ACT queue) / xl (SP queue) in 1MB chunks.
            for i in range(2):
                sl = slice(2 * i, 2 * i + 2)
                nc.scalar.dma_start(out=sq[:, sl, :], in_=sqv[:, sl, :])
            wqh, wkh = load_w("wqh", "wqh"), load_w("wkh", "wkh")
            wql, wkl = load_w("wql", "wql"), load_w("wkl", "wkl")
            for i in range(8):
                sl = slice(2 * i, 2 * i + 2)
                nc.scalar.dma_start(out=xh[:, sl, :], in_=xhv[:, sl, :])
                nc.sync.dma_start(out=xl[:, sl, :], in_=xlv[:, sl, :])
            wvh, wvl = load_w("wvh", "wvh"), load_w("wvl", "wvl")
            woh = singles.tile([128, H_LOC, T], FP8, tag="woh")
            nc.sync.dma_start(out=woh, in_=ap(woh_d).rearrange("(h p) t -> p h t", p=128))
            wol = singles.tile([128, H_LOC, T], FP8, tag="wol")
            nc.sync.dma_start(out=wol, in_=ap(wol_d).rearrange("(h p) t -> p h t", p=128))

            # ---------------- phase 3: projections + attention ----------------
            q_sb = singles.tile([128, H_LOC, T], BF16, tag="q_sb")
            k_sb = singles.tile([128, H_LOC, T], BF16, tag="k_sb")
            v_sb = singles.tile([128, TT, NL], BF16, tag="v_sb")
            vT_sb = singles.tile([128, H_LOC, T], BF16, tag="outT", name="vT_sb")
            outT = singles.tile([128, H_LOC, T], BF16, tag="outT")
            oth = singles.tile([128, H_LOC, T], FP8, tag="oth")
            otl = singles.tile([128, H_LOC, T], FP8, tag="otl")

            def proj_group(ps, wh, wl, hs, js):
                # 3-shot residual fp8 DoubleRow; hi shots first (xl lands last)
                for sh, (wa, xb) in enumerate(((wh, xh), (wl, xh), (wh, xl))):
                    for i in range(KD // 2):
                        kp = slice(2 * i, 2 * i + 2)
                        nc.tensor.matmul(ps, lhsT=wa[:, kp, hs], rhs=xb[:, kp, js],
                                         start=(sh == 0 and i == 0),
                                         stop=(sh == 2 and i == KD // 2 - 1),
                                         perf_mode=DR)

            def emit_vT_strip(h, j):
                hs = slice(h * 128, (h + 1) * 128)
                js = slice(j * 512, (j + 1) * 512)
                ps = pmm.tile([128, 512], F32, tag="mm")
                proj_group(ps, wvh, wvl, hs, js)
                if j % 2 == 0:
                    nc.vector.tensor_copy(vT_sb[:, h, js], ps)
                else:
                    nc.scalar.copy(vT_sb[:, h, js], ps)

            def emit_v_finish(h):
                hs = slice(h * 128, (h + 1) * 128)
                nc.sync.dma_start_transpose(v_sb[:, :, hs], vT_sb[:, h, :])
                for tt in range(TT):
                    nc.vector.tensor_scalar_mul(
                        v_sb[:, tt, hs], v_sb[:, tt, hs], sk_t[:, tt : tt + 1]
                    )

            def emit_rope(ev, dst, h, js):
                # dh-half swap via a PE permutation matmul (PSUM out), so the
                # sin mul is a legal PSUM x SBUF op; sign lives in the table
                m1 = rope.tile([128, 512], BF16, tag="m1")
                nc.vector.tensor_mul(m1, ev, cos_s[:, js])
                msw = psc.tile([128, 512], F32, tag="sc")
                nc.tensor.matmul(msw, lhsT=pswp, rhs=ev, start=True, stop=True)
                m2 = rope.tile([128, 512], BF16, tag="m2")
                nc.vector.tensor_mul(m2, msw, sin_s[:, js])
                nc.vector.tensor_add(dst[:, h, js], m1, m2)

            def emit_qk_group(h, dst, wh, wl, j):
                # projection + immediate ACT evac (frees the PSUM bank fast)
                ps = pmm.tile([128, 512], F32, tag="mm")
                proj_group(ps, wh, wl, slice(h * 128, (h + 1) * 128),
                           slice(j * 512, (j + 1) * 512))
                ev = qke.tile([128, 512], BF16, tag="qke")
                nc.scalar.copy(ev, ps)
                return ev

            def emit_qk_strip(h, j):
                js = slice(j * 512, (j + 1) * 512)
                for dst, wh, wl in ((k_sb, wkh, wkl), (q_sb, wqh, wql)):
                    ev = emit_qk_group(h, dst, wh, wl, j)
                    emit_rope(ev, dst, h, js)

            def emit_attention(h, j):
                hs = slice(h * 128, (h + 1) * 128)
                js = slice(j * 512, (j + 1) * 512)
                ntk = 4 * (j + 1)
                po = ppv.tile([128, 512], F32, tag="pv")
                su = psums.tile([1, 512], F32, tag="su")
                for i in range(ntk):
                    # causal trim: diagonal tile r = i-4j only has valid scores in
                    # columns f >= 128r; restrict every matmul/exp to that range.
                    r = i - 4 * j
                    c0 = 128 * r if r >= 0 else 0
                    cs = slice(c0, 512)           # kept tq columns within strip
                    qs = slice(j * 512 + c0, (j + 1) * 512)
                    st = psc.tile([128, 512], F32, tag="sc")
                    nc.tensor.matmul(
                        st[:, cs], lhsT=k_sb[:, h, i * 128 : (i + 1) * 128],
                        rhs=q_sb[:, h, qs], start=True, stop=(r < 0),
                        skip_group_check=True,
                    )
                    if r >= 0:
                        # causal boundary: -1e30 above the diagonal via a
                        # step-matrix matmul accumulated into the same PSUM
                        nc.tensor.matmul(
                            st[:, c0 : c0 + 128], lhsT=stepm, rhs=negim,
                            start=False, stop=True, skip_group_check=True,
                        )
                    e = epool.tile([128, 512], BF16, tag="e")
                    nc.scalar.activation(e[:, cs], st[:, cs], AF.Exp, scale=INV_SQRT_DH)
                    nc.tensor.matmul(
                        po[:, cs], lhsT=v_sb[:, i, hs], rhs=e[:, cs],
                        start=(i == 0), stop=(i == ntk - 1),
                    )
                    nc.tensor.matmul(
                        su[:, cs], lhsT=on128, rhs=e[:, cs],
                        start=(i == 0), stop=(i == ntk - 1),
                    )
                pos = rbsp.tile([128, 512], BF16, tag="pos")
                lnr = small.tile([1, 512], F32, tag="lnr")
                rec = small.tile([1, 512], F32R, tag="rec")
                rb = psums.tile([128, 512], F32, tag="su")
                # the LAST epilogue is on the tail critical path: run it in
                # halves so the first wo tiles (and their out-DMAs) start early
                nh = 2 if (h == H_LOC - 1 and j == NS - 1) else 1
                w2 = 512 // nh
                for q2 in range(nh):
                    sl = slice(q2 * w2, (q2 + 1) * w2)
                    jsl = slice(j * 512 + q2 * w2, j * 512 + (q2 + 1) * w2)
                    nc.vector.tensor_copy(pos[:, sl], po[:, sl])
                    nc.scalar.activation(lnr[:, sl], su[:, sl], AF.Ln)
                    nc.scalar.activation(rec[:, sl], lnr[:, sl], AF.Exp, scale=-1.0)
                    nc.tensor.matmul(rb[:, sl], lhsT=onc, rhs=rec[:, sl],
                                     start=(q2 == 0), stop=(q2 == nh - 1),
                                     skip_group_check=True)
                    nc.vector.tensor_mul(outT[:, h, jsl], rb[:, sl], pos[:, sl])
                    # residual fp8 split of outT for the DoubleRow wo projection
                    nc.vector.tensor_copy(oth[:, h, jsl], outT[:, h, jsl])
                    nc.vector.tensor_sub(otl[:, h, jsl], outT[:, h, jsl], oth[:, h, jsl])

            def emit_wo_tile(tt):
                ts = slice(tt * 128, (tt + 1) * 128)
                for n in range(NS):
                    ns = slice(n * 512, (n + 1) * 512)
                    ps = pmm.tile([128, 512], F32, tag="mm")
                    for sh, (oa, wb) in enumerate(((oth, woh), (otl, woh), (oth, wol))):
                        nc.tensor.matmul(
                            ps, lhsT=oa[:, :, ts], rhs=wb[:, :, ns],
                            start=(sh == 0), stop=(sh == 2), perf_mode=DR,
                        )
                    stg = stage.tile([128, 512], BF16, tag="stg")
                    if n % 2 == 0:
                        nc.vector.tensor_copy(stg, ps)
                    else:
                        nc.scalar.copy(stg, ps)
                    nc.sync.dma_start(out=out_ap[ts, ns], in_=stg)

            # emission order = scheduler priority: feed PE the next strip's q/k/v
            # before the current strip's epilogue so PE never waits on ACT/DVE.
            # ---------------- phase 1: ssq via DoubleRow ones-matmuls ---------
            ssqp = [psc.tile([32, 512], F32, tag="sc", name=f"ssq{i}") for i in range(2)] + \
                   [pmm.tile([32, 512], F32, tag="mm", name=f"ssq{i+2}") for i in range(2)]
            for i in range(KD // 8):
                kp = slice(2 * i, 2 * i + 2)
                for j in range(NS):
                    nc.tensor.matmul(
                        ssqp[j], lhsT=on8, rhs=sq[:, kp, j * 512 : (j + 1) * 512],
                        start=(i == 0), stop=(i == KD // 8 - 1), perf_mode=DR,
                    )

            combos0 = [(h, dst, wh, wl) for h in range(H_LOC)
                       for dst, wh, wl in ((k_sb, wkh, wkl), (q_sb, wqh, wql))]
            g0 = [(emit_qk_group(h, dst, wh, wl, 0), dst, h)
                  for h, dst, wh, wl in combos0[:3]]
            # ---------------- phase 2: s chain + tables -----------------------
            for j in range(NS):
                js = slice(j * 512, (j + 1) * 512)
                s_m = small.tile([1, 512], F32, tag="lnr")
                nc.vector.tensor_copy(s_m, ssqp[j][0:1, :])
                nc.scalar.activation(s_m, s_m, AF.Ln, bias=epsb, scale=1.0 / D)
                nc.scalar.activation(s_row[:, js], s_m, AF.Exp, bias=lnws, scale=-0.5)
                sb = psc.tile([128, 512], F32, tag="sc")
                nc.tensor.matmul(sb, lhsT=onc, rhs=s_row[:, js], start=True, stop=True)
                cstrip = rope.tile([128, 512], BF16, tag="m1")
                nc.sync.dma_start(out=cstrip, in_=ap(cos_d)[:, js])
                nc.vector.tensor_mul(cos_s[:, js], cstrip, sb)
                sstrip = rope.tile([128, 512], BF16, tag="m2")
                nc.sync.dma_start(out=sstrip, in_=ap(sin_d)[:, js])
                nc.vector.tensor_mul(sin_s[:, js], sstrip, sb)
            # round-trip for the [128, TT] t-tile layout (v scaling)
            nc.sync.dma_start(out=s_scr_ap.rearrange("i p -> () (i p)"), in_=s_row.bitcast(F32))
            nc.sync.dma_start(out=sk_t, in_=s_scr_ap.rearrange("i p -> p i"))

            h4, dst4, wh4, wl4 = combos0[3]
            g0.append((emit_qk_group(h4, dst4, wh4, wl4, 0), dst4, h4))
            for ev, dst, h in g0:
                emit_rope(ev, dst, h, slice(0, 512))
            for h in range(H_LOC):
                for j in range(NS):
                    emit_vT_strip(h, j)
                emit_v_finish(h)
            for j in range(NS):
                for h in range(H_LOC):
                    emit_attention(h, j)
                if j + 1 < NS:
                    for h in range(H_LOC):
                        emit_qk_strip(h, j + 1)
                for tt in range(4 * j, 4 * (j + 1)):
                    emit_wo_tile(tt)

        for _rep in range(repeats):
            emit_body()

    # Force Exp and Ln onto the single combined table set: drop them from
    # every other set in the (cached, order-preserving) table map so the
    # table-load pass picks natural_log_exp_and_others for both — one
    # ACT_TABLE_LOAD for the whole kernel instead of per-strip thrash.
    from concourse.hw_specs import get_activation_tables
    tabs = get_activation_tables(nc.m.arch)
    for nm_, fs_ in tabs.items():
        if nm_ != "natural_log_exp_and_others":
            fs_.discard(AF.Exp)
            fs_.discard(AF.Ln)
    nc.compile()
    _CACHED[repeats] = nc
    return nc


def _host_prep(x, w_ln, wq, wk, wv, wo, cos, sin):
    bf = ml_dtypes.bfloat16
    f8 = ml_dtypes.float8_e4m3
    x = np.asarray(x, np.float32)
    w_ln = np.asarray(w_ln, np.float32)
    cosT = np.ascontiguousarray(np.asarray(cos, np.float32).T).astype(bf)
    sinTf = np.ascontiguousarray(np.asarray(sin, np.float32).T)
    sinTf[0:64] *= -1.0          # rotate_half sign folded into the table
    sinT = sinTf.astype(bf)

    xT = np.ascontiguousarray(x.T)
    xh = xT.astype(f8)
    xl = (xT - xh.astype(np.float32)).astype(f8)
    sq8 = (xT.astype(np.float32) ** 2).reshape(D // 4, 4, T).sum(axis=1).astype(f8)

    # causal step/neg tiles: step.T @ negI adds -1e30 where tq < tk
    p = np.arange(128)
    stepm = (p[:, None] < p[None, :]).astype(bf)
    pswap = np.zeros((128, 128), np.float32)
    pswap[(p + 64) % 128, p] = 1.0          # mswap[i] = ev[(i+64)%128]
    pswap = pswap.astype(bf)
    negim = (-1e30 * np.eye(128)).astype(bf)
    ones_col = np.ones((1, 128), np.float32)
    ones128 = np.ones((128, 1), bf)
    ones8 = np.ones((128, 2, 32), f8)

    wq_s = (np.asarray(wq, np.float32) * w_ln[None, :]) * WS
    wk_s = (np.asarray(wk, np.float32) * w_ln[None, :]) * WS
    wv_s = (np.asarray(wv, np.float32) * w_ln[None, :]) * WS
    wo32 = np.asarray(wo, np.float32)

    def split8(w):                 # [NL, D] f32 -> hi, lo fp8 (transposed)
        wT = np.ascontiguousarray(w.T)
        hi = wT.astype(f8)
        lo = (wT - hi.astype(np.float32)).astype(f8)
        return hi, lo

    in_maps = []
    for c in range(N_CORES):
        sl = slice(c * NL, (c + 1) * NL)
        wqh, wql = split8(wq_s[sl])
        wkh, wkl = split8(wk_s[sl])
        wvh, wvl = split8(wv_s[sl])
        woT = np.ascontiguousarray(wo32[:, sl].T) * WS
        woh_ = woT.astype(f8)
        wol_ = (woT - woh_.astype(np.float32)).astype(f8)
        in_maps.append({
            "xh": xh, "xl": xl, "sq8": sq8,
            "wqh": wqh, "wql": wql,
            "wkh": wkh, "wkl": wkl,
            "wvh": wvh, "wvl": wvl,
            "woh": woh_, "wol": wol_,
            "cosT": cosT, "sinT": sinT,
            "stepm": stepm, "negim": negim, "pswap": pswap,
            "ones_col": ones_col, "ones128": ones128, "ones8": ones8,
        })
    return in_maps


def kernel(x, w_ln, wq, wk, wv, wo, cos, sin):
    nc = _build_program()
    in_maps = _host_prep(x, w_ln, wq, wk, wv, wo, cos, sin)
    t0 = time.time()
    res = run_bass_kernel_spmd(nc, in_maps, core_ids=list(range(N_CORES)))
    t1 = time.time()
    print(f"run_bass_kernel_spmd wall: {(t1 - t0) * 1e3:.1f} ms", file=sys.stderr)
    acc = np.zeros((T, D), np.float32)
    for r in res.results:
        acc += r["out"].astype(np.float32) / WS
    return np.asarray(x, np.float32) + acc


# revision 66
# speedup vs baseline: 1.0161x; 1.0149x over previous
"""Trainium2 Bass kernel for nn_Attention (T=2048, D=2048, H=16, Dh=128).

Strategy: tensor-parallel over heads, 2 heads per core on 8 cores.
  - host folds w_ln into wq/wk/wv, splits x and the (WS=64-scaled) weights
    into fp8(e4m3) hi/lo residual pairs, precomputes fp8 x^2, and folds the
    rotate_half sign into the sin table
  - device: q/k/v projections as 3-shot residual-fp8 DoubleRow matmuls
    (hi@hi + lo@hi + hi@lo over kd-pairs, 0.75x the bf16 cycles); RMSNorm
    sums as DoubleRow fp8 ones-matmuls over the hosted x^2; projection
    PSUMs evacuated to bf16 SBUF on ACT immediately so PSUM banks recycle
    at PE speed; RoPE on DVE from the evacs; causal mask folded into the
    score PSUM via a step-matrix matmul; softmax normalization deferred to
    the PV epilogue via exp(-ln(sum)); per-head output projection in PSUM,
    staged to bf16 and DMA'd out
  - host sums the 8 bf16 partial outputs in f32 and adds the residual x.
"""

import math
import os
import sys
import time

for _p in ("/opt/trn_rl_repo", "/root/.axon_site/_ro/trn_rl_repo"):
    if os.path.isdir(_p) and _p not in sys.path:
        sys.path.insert(0, _p)

import numpy as np
import ml_dtypes

import concourse.bass as bass
import concourse.tile as tile
from concourse import bacc, mybir
from concourse.bass_utils import run_bass_kernel_spmd

BF16 = mybir.dt.bfloat16
FP8 = mybir.dt.float8e4
F32R = mybir.dt.float32r
F32 = mybir.dt.float32
AF = mybir.ActivationFunctionType
DR = mybir.MatmulPerfMode.DoubleRow

T = 2048
D = 2048
N_H = 16
D_H = 128
N_CORES = 8
H_LOC = N_H // N_CORES          # heads per core = 2
NL = H_LOC * D_H                # local head width = 256
KD = D // 128                   # contraction tiles = 16
TT = T // 128                   # t tiles = 16
NS = T // 512                   # 512-wide strips = 4
EPS = 1e-5
INV_SQRT_DH = 1.0 / math.sqrt(D_H)
WS = 64.0                       # fp8 weight scale (undone via the s tables)

_CACHED = {}


def _build_program(repeats=1):
    if repeats in _CACHED:
        return _CACHED[repeats]

    nc = bacc.Bacc("TRN2", target_bir_lowering=False, debug=False, num_devices=N_CORES)

    xh_d = nc.dram_tensor("xh", [D, T], FP8, kind="ExternalInput")
    xl_d = nc.dram_tensor("xl", [D, T], FP8, kind="ExternalInput")
    sq_d = nc.dram_tensor("sq8", [D // 4, T], FP8, kind="ExternalInput")
    w_d = {}
    for nm in ("wqh", "wql", "wkh", "wkl", "wvh", "wvl"):
        w_d[nm] = nc.dram_tensor(nm, [D, NL], FP8, kind="ExternalInput")
    woh_d = nc.dram_tensor("woh", [NL, T], FP8, kind="ExternalInput")
    wol_d = nc.dram_tensor("wol", [NL, T], FP8, kind="ExternalInput")
    cos_d = nc.dram_tensor("cosT", [D_H, T], BF16, kind="ExternalInput")
    sin_d = nc.dram_tensor("sinT", [D_H, T], BF16, kind="ExternalInput")
    step_d = nc.dram_tensor("stepm", [128, 128], BF16, kind="ExternalInput")
    negi_d = nc.dram_tensor("negim", [128, 128], BF16, kind="ExternalInput")
    pswp_d = nc.dram_tensor("pswap", [128, 128], BF16, kind="ExternalInput")
    onc_d = nc.dram_tensor("ones_col", [1, 128], F32R, kind="ExternalInput")
    on128_d = nc.dram_tensor("ones128", [128, 1], BF16, kind="ExternalInput")
    on8_d = nc.dram_tensor("ones8", [128, 2, 32], FP8, kind="ExternalInput")
    out_d = nc.dram_tensor("out", [T, D], BF16, kind="ExternalOutput")
    # DRAM scratch for the s row->tile-layout round trip
    s_scr = nc.dram_tensor("s_scr", [TT, 128], F32, kind="Internal")

    ap = lambda h: h.ap()
    out_ap, s_scr_ap = ap(out_d), ap(s_scr)

    from contextlib import ExitStack

    with tile.TileContext(nc) as tc, ExitStack() as ctx:
        P = ctx.enter_context  # noqa

        singles = P(tc.tile_pool(name="singles", bufs=1))
        rope = P(tc.tile_pool(name="rope", bufs=4))        # [128,512] bf16
        qke = P(tc.tile_pool(name="qke", bufs=4))          # [128,512] bf16 qk evac
        epool = P(tc.tile_pool(name="epool", bufs=7))      # [128,512] bf16 exp tiles
        rbsp = P(tc.tile_pool(name="rbsp", bufs=2))        # [128,512] bf16 pv evac
        small = P(tc.tile_pool(name="small", bufs=2))      # [1,512] smalls
        stage = P(tc.tile_pool(name="stage", bufs=6))      # [128,512] bf16 out staging
        pmm = P(tc.tile_pool(name="pmm", bufs=3, space="PSUM"))
        psc = P(tc.tile_pool(name="psc", bufs=3, space="PSUM"))
        ppv = P(tc.tile_pool(name="ppv", bufs=1, space="PSUM"))
        psums = P(tc.tile_pool(name="psums", bufs=1, space="PSUM"))

        def emit_body():
            # ---------------- phase 0: loads ----------------------------------
            onc = singles.tile([1, 128], F32R, tag="onc")
            nc.sync.dma_start(out=onc, in_=ap(onc_d))
            on128 = singles.tile([128, 1], BF16, tag="on128")
            nc.sync.dma_start(out=on128, in_=ap(on128_d))
            on8 = singles.tile([128, 2, 32], FP8, tag="on8")
            nc.sync.dma_start(out=on8, in_=ap(on8_d))
            stepm = singles.tile([128, 128], BF16, tag="stepm")
            nc.sync.dma_start(out=stepm, in_=ap(step_d))
            negim = singles.tile([128, 128], BF16, tag="negim")
            nc.sync.dma_start(out=negim, in_=ap(negi_d))
            pswp = singles.tile([128, 128], BF16, tag="pswp")
            nc.sync.dma_start(out=pswp, in_=ap(pswp_d))
            epsb = singles.tile([1, 1], F32, tag="epsb")
            nc.vector.memset(epsb, EPS)
            lnws = singles.tile([1, 1], F32, tag="lnws")
            nc.vector.memset(lnws, -math.log(WS))
            s_row = singles.tile([1, T], F32R, tag="srow")
            cos_s = singles.tile([128, T], BF16, tag="cos_s")
            sin_s = singles.tile([128, T], BF16, tag="sin_s")
            sk_t = singles.tile([128, TT], F32, tag="sk")

            sq = singles.tile([128, KD // 4, T], FP8, tag="sq")
            xh = singles.tile([128, KD, T], FP8, tag="xh")
            xl = singles.tile([128, KD, T], FP8, tag="xl")
            sqv = ap(sq_d).rearrange("(n p) t -> p n t", p=128)
            xhv = ap(xh_d).rearrange("(n p) t -> p n t", p=128)
            xlv = ap(xl_d).rearrange("(n p) t -> p n t", p=128)

            def load_w(nm, tag):
                t_ = singles.tile([128, KD, NL], FP8, tag=tag)
                nc.sync.dma_start(out=t_, in_=ap(w_d[nm]).rearrange("(a p) m -> p a m", p=128))
                return t_

            # folded x^2 first (2 x 0.5MB) so the ssq matmuls start early; then
            # the q/k weights, then xh (ACT queue) / xl (SP queue) in 1MB chunks.
            for i in range(2):
                sl = slice(2 * i, 2 * i + 2)
                nc.scalar.dma_start(out=sq[:, sl, :], in_=sqv[:, sl, :])
            wqh, wkh = load_w("wqh", "wqh"), load_w("wkh", "wkh")
            wql, wkl = load_w("wql", "wql"), load_w("wkl", "wkl")
            for i in range(8):
                sl = slice(2 * i, 2 * i + 2)
                nc.scalar.dma_start(out=xh[:, sl, :], in_=xhv[:, sl, :])
                nc.sync.dma_start(out=xl[:, sl, :], in_=xlv[:, sl, :])
            wvh, wvl = load_w("wvh", "wvh"), load_w("wvl", "wvl")
            woh = singles.tile([128, H_LOC, T], FP8, tag="woh")
            nc.sync.dma_start(out=woh, in_=ap(woh_d).rearrange("(h p) t -> p h t", p=128))
            wol = singles.tile([128, H_LOC, T], FP8, tag="wol")
            nc.sync.dma_start(out=wol, in_=ap(wol_d).rearrange("(h p) t -> p h t", p=128))

            # ---------------- phase 3: projections + attention ----------------
            q_sb = singles.tile([128, H_LOC, T], BF16, tag="q_sb")
            k_sb = singles.tile([128, H_LOC, T], BF16, tag="k_sb")
            v_sb = singles.tile([128, TT, NL], BF16, tag="v_sb")
            vT_sb = singles.tile([128, H_LOC, T], BF16, tag="outT", name="vT_sb")
            outT = singles.tile([128, H_LOC, T], BF16, tag="outT")
            oth = singles.tile([128, H_LOC, T], FP8, tag="oth")
            otl = singles.tile([128, H_LOC, T], FP8, tag="otl")

            def proj_group(ps, wh, wl, hs, js):
                # 3-shot residual fp8 DoubleRow; hi shots first (xl lands last)
                for sh, (wa, xb) in enumerate(((wh, xh), (wl, xh), (wh, xl))):
                    for i in range(KD // 2):
                        kp = slice(2 * i, 2 * i + 2)
                        nc.tensor.matmul(ps, lhsT=wa[:, kp, hs], rhs=xb[:, kp, js],
                                         start=(sh == 0 and i == 0),
                                         stop=(sh == 2 and i == KD // 2 - 1),
                                         perf_mode=DR)

            def emit_vT_strip(h, j):
                hs = slice(h * 128, (h + 1) * 128)
                js = slice(j * 512, (j + 1) * 512)
                ps = pmm.tile([128, 512], F32, tag="mm")
                proj_group(ps, wvh, wvl, hs, js)
                if j % 2 == 0:
                    nc.vector.tensor_copy(vT_sb[:, h, js], ps)
                else:
                    nc.scalar.copy(vT_sb[:, h, js], ps)

            def emit_v_finish(h):
                hs = slice(h * 128, (h + 1) * 128)
                nc.sync.dma_start_transpose(v_sb[:, :, hs], vT_sb[:, h, :])
                for tt in range(TT):
                    nc.vector.tensor_scalar_mul(
                        v_sb[:, tt, hs], v_sb[:, tt, hs], sk_t[:, tt : tt + 1]
                    )

            def emit_rope(ev, dst, h, js):
                # dh-half swap via a PE permutation matmul (PSUM out), so the
                # sin mul is a legal PSUM x SBUF op; sign lives in the table
                m1 = rope.tile([128, 512], BF16, tag="m1")
                nc.vector.tensor_mul(m1, ev, cos_s[:, js])
                msw = psc.tile([128, 512], F32, tag="sc")
                nc.tensor.matmul(msw, lhsT=pswp, rhs=ev, start=True, stop=True)
                m2 = rope.tile([128, 512], BF16, tag="m2")
                nc.vector.tensor_mul(m2, msw, sin_s[:, js])
                nc.vector.tensor_add(dst[:, h, js], m1, m2)

            def emit_qk_group(h, dst, wh, wl, j):
                # projection + immediate ACT evac (frees the PSUM bank fast)
                ps = pmm.tile([128, 512], F32, tag="mm")
                proj_group(ps, wh, wl, slice(h * 128, (h + 1) * 128),
                           slice(j * 512, (j + 1) * 512))
                ev = qke.tile([128, 512], BF16, tag="qke")
                nc.scalar.copy(ev, ps)
                return ev

            def emit_qk_strip(h, j):
                js = slice(j * 512, (j + 1) * 512)
                for dst, wh, wl in ((k_sb, wkh, wkl), (q_sb, wqh, wql)):
                    ev = emit_qk_group(h, dst, wh, wl, j)
                    emit_rope(ev, dst, h, js)

            def emit_attention(h, j):
                hs = slice(h * 128, (h + 1) * 128)
                js = slice(j * 512, (j + 1) * 512)
                ntk = 4 * (j + 1)
                po = ppv.tile([128, 512], F32, tag="pv")
                su = psums.tile([1, 512], F32, tag="su")
                for i in range(ntk):
                    # causal trim: diagonal tile r = i-4j only has valid scores in
                    # columns f >= 128r; restrict every matmul/exp to that range.
                    r = i - 4 * j
                    c0 = 128 * r if r >= 0 else 0
                    cs = slice(c0, 512)           # kept tq columns within strip
                    qs = slice(j * 512 + c0, (j + 1) * 512)
                    st = psc.tile([128, 512], F32, tag="sc")
                    nc.tensor.matmul(
                        st[:, cs], lhsT=k_sb[:, h, i * 128 : (i + 1) * 128],
                        rhs=q_sb[:, h, qs], start=True, stop=(r < 0),
                        skip_group_check=True,
                    )
                    if r >= 0:
                        # causal boundary: -1e30 above the diagonal via a
                        # step-matrix matmul accumulated into the same PSUM
                        nc.tensor.matmul(
                            st[:, c0 : c0 + 128], lhsT=stepm, rhs=negim,
                            start=False, stop=True, skip_group_check=True,
                        )
                    e = epool.tile([128, 512], BF16, tag="e")
                    nc.scalar.activation(e[:, cs], st[:, cs], AF.Exp, scale=INV_SQRT_DH)
                    nc.tensor.matmul(
                        po[:, cs], lhsT=v_sb[:, i, hs], rhs=e[:, cs],
                        start=(i == 0), stop=(i == ntk - 1),
                    )
                    nc.tensor.matmul(
                        su[:, cs], lhsT=on128, rhs=e[:, cs],
                        start=(i == 0), stop=(i == ntk - 1),
                    )
                pos = rbsp.tile([128, 512], BF16, tag="pos")
                lnr = small.tile([1, 512], F32, tag="lnr")
                rec = small.tile([1, 512], BF16, tag="rec8")
                rbs = rbsp.tile([128, 512], BF16, tag="rbs")
                # normalization broadcast on the idle Pool engine (frees the
                # psums bank immediately after the Ln read, so the next head's
                # su accumulation never waits on this epilogue)
                nh = 2 if (h == H_LOC - 1 and j == NS - 1) else 1
                w2 = 512 // nh
                for q2 in range(nh):
                    sl = slice(q2 * w2, (q2 + 1) * w2)
                    jsl = slice(j * 512 + q2 * w2, j * 512 + (q2 + 1) * w2)
                    nc.vector.tensor_copy(pos[:, sl], po[:, sl])
                    nc.scalar.activation(lnr[:, sl], su[:, sl], AF.Ln)
                    nc.scalar.activation(rec[:, sl], lnr[:, sl], AF.Exp, scale=-1.0)
                    nc.gpsimd.partition_broadcast(rbs[:, sl], rec[:, sl])
                    nc.vector.tensor_mul(outT[:, h, jsl], rbs[:, sl], pos[:, sl])
                    # residual fp8 split of outT for the DoubleRow wo projection
                    nc.vector.tensor_copy(oth[:, h, jsl], outT[:, h, jsl])
                    nc.vector.tensor_sub(otl[:, h, jsl], outT[:, h, jsl], oth[:, h, jsl])

            def emit_wo_tile(tt):
                ts = slice(tt * 128, (tt + 1) * 128)
                for n in range(NS):
                    ns = slice(n * 512, (n + 1) * 512)
                    ps = pmm.tile([128, 512], F32, tag="mm")
                    for sh, (oa, wb) in enumerate(((oth, woh), (otl, woh), (oth, wol))):
                        nc.tensor.matmul(
                            ps, lhsT=oa[:, :, ts], rhs=wb[:, :, ns],
                            start=(sh == 0), stop=(sh == 2), perf_mode=DR,
                        )
                    stg = stage.tile([128, 512], BF16, tag="stg")
                    if n % 2 == 0:
                        nc.vector.tensor_copy(stg, ps)
                    else:
                        nc.scalar.copy(stg, ps)
                    nc.sync.dma_start(out=out_ap[ts, ns], in_=stg)

            # emission order = scheduler priority: feed PE the next strip's q/k/v
            # before the current strip's epilogue so PE never waits on ACT/DVE.
            # ---------------- phase 1: ssq via DoubleRow ones-matmuls ---------
            ssqp = [psc.tile([32, 512], F32, tag="sc", name=f"ssq{i}") for i in range(2)] + \
                   [pmm.tile([32, 512], F32, tag="mm", name=f"ssq{i+2}") for i in range(2)]
            for i in range(KD // 8):
                kp = slice(2 * i, 2 * i + 2)
                for j in range(NS):
                    nc.tensor.matmul(
                        ssqp[j], lhsT=on8, rhs=sq[:, kp, j * 512 : (j + 1) * 512],
                        start=(i == 0), stop=(i == KD // 8 - 1), perf_mode=DR,
                    )

            combos0 = [(h, dst, wh, wl) for h in range(H_LOC)
                       for dst, wh, wl in ((k_sb, wkh, wkl), (q_sb, wqh, wql))]
            g0 = [(emit_qk_group(h, dst, wh, wl, 0), dst, h)
                  for h, dst, wh, wl in combos0[:3]]
            # ---------------- phase 2: s chain + tables -----------------------
            for j in range(NS):
                js = slice(j * 512, (j + 1) * 512)
                s_m = small.tile([1, 512], F32, tag="lnr")
                nc.vector.tensor_copy(s_m, ssqp[j][0:1, :])
                nc.scalar.activation(s_m, s_m, AF.Ln, bias=epsb, scale=1.0 / D)
                nc.scalar.activation(s_row[:, js], s_m, AF.Exp, bias=lnws, scale=-0.5)
                sb = psc.tile([128, 512], F32, tag="sc")
                nc.tensor.matmul(sb, lhsT=onc, rhs=s_row[:, js], start=True, stop=True)
                cstrip = rope.tile([128, 512], BF16, tag="m1")
                nc.sync.dma_start(out=cstrip, in_=ap(cos_d)[:, js])
                nc.vector.tensor_mul(cos_s[:, js], cstrip, sb)
                sstrip = rope.tile([128, 512], BF16, tag="m2")
                nc.sync.dma_start(out=sstrip, in_=ap(sin_d)[:, js])
                nc.vector.tensor_mul(sin_s[:, js], sstrip, sb)
            # round-trip for the [128, TT] t-tile layout (v scaling)
            nc.sync.dma_start(out=s_scr_ap.rearrange("i p -> () (i p)"), in_=s_row.bitcast(F32))
            nc.sync.dma_start(out=sk_t, in_=s_scr_ap.rearrange("i p -> p i"))

            h4, dst4, wh4, wl4 = combos0[3]
            g0.append((emit_qk_group(h4, dst4, wh4, wl4, 0), dst4, h4))
            for ev, dst, h in g0:
                emit_rope(ev, dst, h, slice(0, 512))
            for h in range(H_LOC):
                for j in range(NS):
                    emit_vT_strip(h, j)
                emit_v_finish(h)
            for j in range(NS):
                for h in range(H_LOC):
                    emit_attention(h, j)
                if j + 1 < NS:
                    for h in range(H_LOC):
                        emit_qk_strip(h, j + 1)
                for tt in range(4 * j, 4 * (j + 1)):
                    emit_wo_tile(tt)

        for _rep in range(repeats):
            emit_body()

    # Force Exp and Ln onto the single combined table set: drop them from
    # every other set in the (cached, order-preserving) table map so the
    # table-load pass picks natural_log_exp_and_others for both — one
    # ACT_TABLE_LOAD for the whole kernel instead of per-strip thrash.
    from concourse.hw_specs import get_activation_tables
    tabs = get_activation_tables(nc.m.arch)
    for nm_, fs_ in tabs.items():
        if nm_ != "natural_log_exp_and_others":
            fs_.discard(AF.Exp)
            fs_.discard(AF.Ln)
    nc.compile()
    _CACHED[repeats] = nc
    return nc


def _host_prep(x, w_ln, wq, wk, wv, wo, cos, sin):
    bf = ml_dtypes.bfloat16
    f8 = ml_dtypes.float8_e4m3
    x = np.asarray(x, np.float32)
    w_ln = np.asarray(w_ln, np.float32)
    cosT = np.ascontiguousarray(np.asarray(cos, np.float32).T).astype(bf)
    sinTf = np.ascontiguousarray(np.asarray(sin, np.float32).T)
    sinTf[0:64] *= -1.0          # rotate_half sign folded into the table
    sinT = sinTf.astype(bf)

    xT = np.ascontiguousarray(x.T)
    xh = xT.astype(f8)
    xl = (xT - xh.astype(np.float32)).astype(f8)
    sq8 = (xT.astype(np.float32) ** 2).reshape(D // 4, 4, T).sum(axis=1).astype(f8)

    # causal step/neg tiles: step.T @ negI adds -1e30 where tq < tk
    p = np.arange(128)
    stepm = (p[:, None] < p[None, :]).astype(bf)
    pswap = np.zeros((128, 128), np.float32)
    pswap[(p + 64) % 128, p] = 1.0          # mswap[i] = ev[(i+64)%128]
    pswap = pswap.astype(bf)
    negim = (-1e30 * np.eye(128)).astype(bf)
    ones_col = np.ones((1, 128), np.float32)
    ones128 = np.ones((128, 1), bf)
    ones8 = np.ones((128, 2, 32), f8)

    wq_s = (np.asarray(wq, np.float32) * w_ln[None, :]) * WS
    wk_s = (np.asarray(wk, np.float32) * w_ln[None, :]) * WS
    wv_s = (np.asarray(wv, np.float32) * w_ln[None, :]) * WS
    wo32 = np.asarray(wo, np.float32)

    def split8(w):                 # [NL, D] f32 -> hi, lo fp8 (transposed)
        wT = np.ascontiguousarray(w.T)
        hi = wT.astype(f8)
        lo = (wT - hi.astype(np.float32)).astype(f8)
        return hi, lo

    in_maps = []
    for c in range(N_CORES):
        sl = slice(c * NL, (c + 1) * NL)
        wqh, wql = split8(wq_s[sl])
        wkh, wkl = split8(wk_s[sl])
        wvh, wvl = split8(wv_s[sl])
        woT = np.ascontiguousarray(wo32[:, sl].T) * WS
        woh_ = woT.astype(f8)
        wol_ = (woT - woh_.astype(np.float32)).astype(f8)
        in_maps.append({
            "xh": xh, "xl": xl, "sq8": sq8,
            "wqh": wqh, "wql": wql,
            "wkh": wkh, "wkl": wkl,
            "wvh": wvh, "wvl": wvl,
            "woh": woh_, "wol": wol_,
            "cosT": cosT, "sinT": sinT,
            "stepm": stepm, "negim": negim, "pswap": pswap,
            "ones_col": ones_col, "ones128": ones128, "ones8": ones8,
        })
    return in_maps


def kernel(x, w_ln, wq, wk, wv, wo, cos, sin):
    nc = _build_program()
    in_maps = _host_prep(x, w_ln, wq, wk, wv, wo, cos, sin)
    t0 = time.time()
    res = run_bass_kernel_spmd(nc, in_maps, core_ids=list(range(N_CORES)))
    t1 = time.time()
    print(f"run_bass_kernel_spmd wall: {(t1 - t0) * 1e3:.1f} ms", file=sys.stderr)
    acc = np.zeros((T, D), np.float32)
    for r in res.results:
        acc += r["out"].astype(np.float32) / WS
    return np.asarray(x, np.float32) + acc
